# revision 35
# baseline (speedup 1.0000x reference)
"""Tensor-parallel GQA attention (RoPE + causal softmax) on 8 TRN2 NeuronCores.

Sharding: wq/wk/wv column-sharded by head (4 q heads + 2 kv heads per core),
wo row-sharded; x replicated. Each core computes a partial [4096, 2048]
(transposed) output; the host sums the 8 partials and transposes.

All on-device layouts keep the contraction dim on SBUF partitions:
  - projections:  psum[m,s]   = sum_k  W[k,m].T   @ xT[k,s]      (q/k/v in ^T layout)
  - RoPE: host de-interleaves wq/wk columns per head (evens then odds) so the
    rotation becomes full-lane DVE ops + one SBUF->SBUF partition-half swap DMA.
  - scores^T[k,q] = kT[d,k].T @ qT[d,q];  E = exp(scale*s) on ACT, causal mask
    as a 0/1 multiply on (diagonal) partial blocks only.
  - out^T[d',q] += v[k,d'].T @ E[k,q]  accumulated over k-blocks (PSUM), with a
    parallel ones-stationary matmul producing Z[1,q]; normalization is deferred
    off the critical path (fast reciprocal + partition broadcast + in-place mul).
  - final^T[o,q] += wo[d',o].T @ attnT[d',q].

The whole kernel is chunk-pipelined: for each 512-token chunk c,
A(c) projection -> B(c) attention -> C(c) output projection, so the three
phases overlap across chunks.
"""

import numpy as np
import ml_dtypes

import concourse.bacc as bacc
import concourse.bass as bass
import concourse.mybir as mybir
import concourse.tile as tile
from concourse.bass_utils import run_bass_kernel_spmd

BF16 = mybir.dt.bfloat16
F32 = mybir.dt.float32
NPBF16 = ml_dtypes.bfloat16

EMBED = 4096
S = 2048
HEADS = 32
KV_HEADS = 16
HD = 128
N_CORES = 8
Q_PER_CORE = HEADS // N_CORES          # 4
KV_PER_CORE = KV_HEADS // N_CORES      # 2
QDIM = Q_PER_CORE * HD                 # 512
KVDIM = KV_PER_CORE * HD               # 256
KT = EMBED // 128                      # 32 contraction k-tiles
SCH = 512                              # s-chunk (matmul moving cols / PSUM bank)
NSC = S // SCH                         # 4 s-chunks
NKB = S // 128                         # 16 k-blocks
M_TILES = Q_PER_CORE + KV_PER_CORE + KV_PER_CORE  # 8 projection m-tiles
NOT = EMBED // 128                     # 32 output tiles
SCALE = 1.0 / float(np.sqrt(HD))

_cache = {}


def _build():
    nc = bacc.Bacc("TRN2", target_bir_lowering=False, debug=False,
                   num_devices=N_CORES)

    # m-tile-major / chunk-major layouts so every DMA is contiguous per partition
    xT = nc.dram_tensor("xT", [NSC, 128, KT, SCH], BF16, kind="ExternalInput")
    wqkv = nc.dram_tensor("wqkv", [M_TILES, 128, KT, 128], BF16, kind="ExternalInput")
    wo = nc.dram_tensor("wo", [NOT, 128, Q_PER_CORE, 128], BF16, kind="ExternalInput")
    cs = nc.dram_tensor("cs", [128, S], F32, kind="ExternalInput")
    sn = nc.dram_tensor("sn", [128, S], F32, kind="ExternalInput")
    masks = nc.dram_tensor("masks", [128, 128], BF16, kind="ExternalInput")
    ident = nc.dram_tensor("ident", [128, 128], BF16, kind="ExternalInput")
    out = nc.dram_tensor("out", [EMBED, S], F32, kind="ExternalOutput")

    with tile.TileContext(nc) as tc:
        with (
            tc.tile_pool(name="const", bufs=1) as constp,
            tc.tile_pool(name="persist", bufs=1) as persist,
            tc.tile_pool(name="wstream", bufs=4) as wpool,
            tc.tile_pool(name="xc", bufs=2) as xcpool,
            tc.tile_pool(name="stg", bufs=2) as stg,
            tc.tile_pool(name="epool", bufs=4) as epool,
            tc.tile_pool(name="odrain", bufs=2) as odrain,
            tc.tile_pool(name="bcast", bufs=1) as bcpool,
            tc.tile_pool(name="small", bufs=2) as smallp,
            tc.tile_pool(name="wostream", bufs=3) as wopool,
            tc.tile_pool(name="proj_ps", bufs=2, space=bass.MemorySpace.PSUM) as proj_ps,
            tc.tile_pool(name="s_ps", bufs=2, space=bass.MemorySpace.PSUM) as s_ps,
            tc.tile_pool(name="o_ps", bufs=1, space=bass.MemorySpace.PSUM) as o_ps,
            tc.tile_pool(name="misc_ps", bufs=1, space=bass.MemorySpace.PSUM) as misc_ps,
        ):
            # ---- persistent activations (declared first so DMAs below can
            # target them) ----
            qkT = persist.tile([128, Q_PER_CORE + KV_PER_CORE, S], BF16)
            vt = persist.tile([128, NKB, KVDIM], BF16)
            attn = persist.tile([128, Q_PER_CORE, S], BF16)

            # ---- constants ----
            cs_t = constp.tile([128, S], F32)
            sn_t = constp.tile([128, S], F32)
            mask_t = constp.tile([128, 128], BF16)
            ident_t = constp.tile([128, 128], BF16)
            ones_t = constp.tile([128, 1], BF16)
            nc.gpsimd.dma_start(out=cs_t[:], in_=cs[:])
            nc.gpsimd.dma_start(out=sn_t[:], in_=sn[:])
            nc.gpsimd.dma_start(out=mask_t[:], in_=masks[:])
            nc.gpsimd.dma_start(out=ident_t[:], in_=ident[:])
            nc.vector.memset(ones_t[:], 1.0)

            for c in range(NSC):
                c0, c1 = c * SCH, (c + 1) * SCH

                # ============ A(c): QKV projection + RoPE + V transpose
                xc = xcpool.tile([128, KT, SCH], BF16)
                for piece in range(4):
                    kt0 = piece * (KT // 4)
                    kt1 = kt0 + KT // 4
                    nc.gpsimd.dma_start(
                        out=xc[:, kt0:kt1, :],
                        in_=xT[c, :, kt0:kt1, :])
                # m-tile order: k0 and q0 first so attention on head 0 can
                # start early; weight DMAs issue from the near-empty gpsimd
                # stream so they never queue behind exp/rope work
                morder = [4, 0, 5, 6, 7, 1, 2, 3]
                wts = {}
                for m in morder[:3]:
                    wt = wpool.tile([128, KT, 128], BF16, tag="wt")
                    nc.gpsimd.dma_start(out=wt[:], in_=wqkv[m])
                    wts[m] = wt
                for mi, m in enumerate(morder):
                    if mi + 3 < M_TILES:
                        mpre = morder[mi + 3]
                        wt = wpool.tile([128, KT, 128], BF16, tag="wt")
                        nc.gpsimd.dma_start(out=wt[:], in_=wqkv[mpre])
                        wts[mpre] = wt
                    wt = wts[m]
                    ps = proj_ps.tile([128, SCH], F32, tag="mm")
                    for k in range(KT):
                        nc.tensor.matmul(ps[:], wt[:, k, :], xc[:, k, :],
                                         start=(k == 0), stop=(k == KT - 1))
                    if m < Q_PER_CORE + KV_PER_CORE:
                        # RoPE: rot = T*cs + swap(T*sn)
                        t1 = stg.tile([128, SCH], F32, tag="t1")
                        t2 = stg.tile([128, SCH], F32, tag="t2")
                        t2s = stg.tile([128, SCH], F32, tag="t2s")
                        nc.vector.tensor_mul(t1[:], ps[:], cs_t[:, c0:c1])
                        nc.vector.tensor_mul(t2[:], ps[:], sn_t[:, c0:c1])
                        nc.sync.dma_start(out=t2s[0:64, :], in_=t2[64:128, :])
                        nc.sync.dma_start(out=t2s[64:128, :], in_=t2[0:64, :])
                        nc.vector.tensor_add(qkT[:, m, c0:c1], t1[:], t2s[:])
                    else:
                        # V: drain to bf16, then DMA-transpose 128x128 blocks
                        kv = m - Q_PER_CORE - KV_PER_CORE
                        vtmp = stg.tile([128, SCH], BF16, tag="vtmp")
                        nc.scalar.activation(vtmp[:], ps[:],
                                             mybir.ActivationFunctionType.Copy)
                        for b in range(SCH // 128):
                            nc.sync.dma_start_transpose(
                                out=vt[:, c * (SCH // 128) + b,
                                       kv * 128:(kv + 1) * 128],
                                in_=vtmp[:, b * 128:(b + 1) * 128])

                # ============ B(c): attention for all 4 heads on q-chunk c
                nkb = (c + 1) * (SCH // 128)  # causal: k-blocks 0..4c+3
                for h in range(Q_PER_CORE):
                    kv = h // 2
                    kvm = Q_PER_CORE + kv
                    po = o_ps.tile([128, SCH], F32)
                    pz = misc_ps.tile([1, SCH], F32, tag="z")
                    es = [None] * nkb
                    # diagonal blocks only touch q-columns >= r0 = 128*off
                    # (columns below r0 are fully causally masked)
                    r0s = [max(0, 128 * (kb - (nkb - (SCH // 128)))) for kb in range(nkb)]
                    # software-pipelined: scores/exp run two k-blocks ahead of PV/Z
                    DEPTH = 2

                    def pv(kb):
                        r0 = r0s[kb]
                        nc.tensor.matmul(po[:, r0:SCH],
                                         vt[:, kb, kv * 128:(kv + 1) * 128],
                                         es[kb][:, r0:SCH],
                                         start=(kb == 0), stop=(kb == nkb - 1),
                                         skip_group_check=True)
                        nc.tensor.matmul(pz[:, r0:], ones_t[:], es[kb][:, r0:SCH],
                                         start=(kb == 0), stop=(kb == nkb - 1),
                                         skip_group_check=True)
                        es[kb] = None

                    for kb in range(nkb):
                        r0 = r0s[kb]
                        pss = s_ps.tile([128, SCH], F32)
                        nc.tensor.matmul(pss[:, r0:SCH],
                                         qkT[:, kvm, kb * 128:(kb + 1) * 128],
                                         qkT[:, h, c0 + r0:c1],
                                         start=True, stop=True)
                        e = epool.tile([128, SCH], BF16)
                        nc.scalar.activation(e[:, r0:], pss[:, r0:SCH],
                                             mybir.ActivationFunctionType.Exp,
                                             scale=SCALE)
                        if kb >= nkb - (SCH // 128):
                            # triangular mask on the diagonal 128-col sub-block
                            nc.vector.tensor_mul(e[:, r0:r0 + 128],
                                                 e[:, r0:r0 + 128], mask_t[:])
                        es[kb] = e
                        if kb >= DEPTH:
                            pv(kb - DEPTH)
                    for kb in range(max(0, nkb - DEPTH), nkb):
                        pv(kb)
                    # drain quickly; normalization is deferred off the PE path
                    nc.scalar.activation(attn[:, h, c0:c1], po[:],
                                         mybir.ActivationFunctionType.Copy)
                    zrow = smallp.tile([1, SCH], F32, tag="zrow")
                    nc.scalar.activation(zrow[:], pz[:],
                                         mybir.ActivationFunctionType.Copy)
                    rec = smallp.tile([1, SCH], F32, tag="rec")
                    nc.vector.reciprocal_approx_fast(rec[:], zrow[:])
                    bc = bcpool.tile([128, SCH], F32)
                    nc.gpsimd.partition_broadcast(bc[:], rec[:])
                    nc.vector.tensor_mul(attn[:, h, c0:c1], attn[:, h, c0:c1], bc[:])

                # ============ C(c): output projection for chunk c
                # ot tiles processed in pairs into a double-wide PSUM tile so
                # the drain runs half as often (drain cadence gated PE here)
                for otp in range(NOT // 2):
                    wot = wopool.tile([128, 2, Q_PER_CORE, 128], BF16)
                    nc.gpsimd.dma_start(out=wot[:, 0], in_=wo[2 * otp])
                    nc.gpsimd.dma_start(out=wot[:, 1], in_=wo[2 * otp + 1])
                    pso = proj_ps.tile([128, 2, SCH], F32, tag="mm")
                    for half in range(2):
                        for d in range(Q_PER_CORE):
                            nc.tensor.matmul(pso[:, half, :], wot[:, half, d, :],
                                             attn[:, d, c0:c1],
                                             start=(d == 0),
                                             stop=(d == Q_PER_CORE - 1))
                    od = odrain.tile([128, 2, SCH], F32)
                    nc.vector.tensor_copy(od[:], pso[:])
                    for half in range(2):
                        ot = 2 * otp + half
                        nc.sync.dma_start(out=out[ot * 128:(ot + 1) * 128, c0:c1],
                                          in_=od[:, half, :])

    nc.compile()
    return nc


def _host_prep(x, freqs_cis, wq, wk, wv, wo, start_pos):
    """Build per-core in_maps (host-side shard + layout transform)."""
    sp = int(start_pos)
    x2 = np.asarray(x, np.float32).reshape(S, EMBED)
    fc = np.asarray(freqs_cis, np.float32)[sp:sp + S]  # [S, 64, 2]
    cos = fc[:, :, 0].T.copy()  # [64, S]
    sin = fc[:, :, 1].T.copy()
    cs = np.concatenate([cos, cos], axis=0).astype(np.float32)          # [128,S]
    sn = np.concatenate([sin, -sin], axis=0).astype(np.float32)         # [128,S]

    # de-interleave perm within each head (evens then odds)
    perm = np.concatenate([np.arange(0, HD, 2), np.arange(1, HD, 2)])
    qperm = np.concatenate([h * HD + perm for h in range(HEADS)])
    kperm = np.concatenate([h * HD + perm for h in range(KV_HEADS)])

    wq_p = np.asarray(wq, np.float32)[:, qperm]
    wk_p = np.asarray(wk, np.float32)[:, kperm]
    wv_p = np.asarray(wv, np.float32)
    wo_p = np.asarray(wo, np.float32)

    xT = np.ascontiguousarray(x2.T).astype(NPBF16)        # [EMBED, S]
    # chunk-major: [NSC, 128, KT*SCH]
    xT4 = np.ascontiguousarray(
        xT.reshape(KT, 128, NSC, SCH).transpose(2, 1, 0, 3))

    kk = np.arange(128)[:, None]
    qq = np.arange(128)[None, :]
    m4 = (qq >= kk).astype(NPBF16)  # [128,128] lower-triangular in q>=k sense

    ident = np.eye(128, dtype=NPBF16)

    in_maps = []
    for c in range(N_CORES):
        wq_c = wq_p[:, c * QDIM:(c + 1) * QDIM]
        wk_c = wk_p[:, c * KVDIM:(c + 1) * KVDIM]
        wv_c = wv_p[:, c * KVDIM:(c + 1) * KVDIM]
        wcat = np.concatenate([wq_c, wk_c, wv_c], axis=1)  # [EMBED, 1024]
        # m-tile-major: [8, 128, KT*128]
        wqkv = np.ascontiguousarray(
            wcat.reshape(KT, 128, M_TILES, 128).transpose(2, 1, 0, 3)
        ).astype(NPBF16)
        wo_c = wo_p[c * QDIM:(c + 1) * QDIM, :]            # [512, EMBED]
        # ot-major: [NOT, 128, QDIM] with [d' within head (128), head (4)] per ot
        woL = np.ascontiguousarray(
            wo_c.reshape(Q_PER_CORE, 128, NOT, 128).transpose(2, 1, 0, 3)
        ).astype(NPBF16)
        in_maps.append({
            "xT": xT4,
            "wqkv": wqkv,
            "wo": woL,
            "cs": cs, "sn": sn, "masks": m4, "ident": ident,
        })
    return in_maps


def kernel(x, freqs_cis, wq, wk, wv, wo, start_pos):
    if "nc" not in _cache:
        _cache["nc"] = _build()
    nc = _cache["nc"]
    in_maps = _host_prep(x, freqs_cis, wq, wk, wv, wo, start_pos)
    res = run_bass_kernel_spmd(nc, in_maps, core_ids=list(range(N_CORES)))
    parts = [res.results[c]["out"] for c in range(N_CORES)]
    total = parts[0].astype(np.float32)
    for p in parts[1:]:
        total = total + p.astype(np.float32)
    return np.ascontiguousarray(total.T).reshape(1, S, EMBED).astype(np.float32)


# revision 39
# speedup vs baseline: 1.2032x; 1.2032x over previous
"""Tensor-parallel GQA attention (RoPE + causal softmax) on 8 TRN2 NeuronCores.

Sharding: wq/wk/wv column-sharded by head (4 q heads + 2 kv heads per core),
wo row-sharded; x replicated. Each core computes a partial [4096, 2048]
(transposed) output; the host sums the 8 partials and transposes.

All on-device layouts keep the contraction dim on SBUF partitions:
  - projections:  psum[m,s]   = sum_k  W[k,m].T   @ xT[k,s]      (q/k/v in ^T layout)
  - RoPE: host de-interleaves wq/wk columns per head (evens then odds) so the
    rotation becomes full-lane DVE ops + one SBUF->SBUF partition-half swap DMA.
  - scores^T[k,q] = kT[d,k].T @ qT[d,q];  E = exp(scale*s) on ACT, causal mask
    as a 0/1 multiply on (diagonal) partial blocks only.
  - out^T[d',q] += v[k,d'].T @ E[k,q]  accumulated over k-blocks (PSUM), with a
    parallel ones-stationary matmul producing Z[1,q]; normalization is deferred
    off the critical path (fast reciprocal + partition broadcast + in-place mul).
  - final^T[o,q] += wo[d',o].T @ attnT[d',q].

The whole kernel is chunk-pipelined: for each 512-token chunk c,
A(c) projection -> B(c) attention -> C(c) output projection, so the three
phases overlap across chunks.
"""

import numpy as np
import ml_dtypes

import concourse.bacc as bacc
import concourse.bass as bass
import concourse.mybir as mybir
import concourse.tile as tile
from concourse.bass_utils import run_bass_kernel_spmd

BF16 = mybir.dt.bfloat16
F32 = mybir.dt.float32
NPBF16 = ml_dtypes.bfloat16

EMBED = 4096
S = 2048
HEADS = 32
KV_HEADS = 16
HD = 128
N_CORES = 8
Q_PER_CORE = HEADS // N_CORES          # 4
KV_PER_CORE = KV_HEADS // N_CORES      # 2
QDIM = Q_PER_CORE * HD                 # 512
KVDIM = KV_PER_CORE * HD               # 256
KT = EMBED // 128                      # 32 contraction k-tiles
SCH = 512                              # s-chunk (matmul moving cols / PSUM bank)
NSC = S // SCH                         # 4 s-chunks
NKB = S // 128                         # 16 k-blocks
M_TILES = Q_PER_CORE + KV_PER_CORE + KV_PER_CORE  # 8 projection m-tiles
NOT = EMBED // 128                     # 32 output tiles
SCALE = 1.0 / float(np.sqrt(HD))

_cache = {}


def _build():
    nc = bacc.Bacc("TRN2", target_bir_lowering=False, debug=False,
                   num_devices=N_CORES)

    # m-tile-major / chunk-major layouts so every DMA is contiguous per partition
    xT = nc.dram_tensor("xT", [NSC, 128, KT, SCH], BF16, kind="ExternalInput")
    wqkv = nc.dram_tensor("wqkv", [M_TILES, 128, KT, 128], BF16, kind="ExternalInput")
    wo = nc.dram_tensor("wo", [NOT, 128, Q_PER_CORE, 128], BF16, kind="ExternalInput")
    cs = nc.dram_tensor("cs", [128, S], F32, kind="ExternalInput")
    sn = nc.dram_tensor("sn", [128, S], F32, kind="ExternalInput")
    masks = nc.dram_tensor("masks", [128, 128], BF16, kind="ExternalInput")
    ident = nc.dram_tensor("ident", [128, 128], BF16, kind="ExternalInput")
    out = nc.dram_tensor("out", [EMBED, S], BF16, kind="ExternalOutput")

    with tile.TileContext(nc) as tc:
        with (
            tc.tile_pool(name="const", bufs=1) as constp,
            tc.tile_pool(name="persist", bufs=1) as persist,
            tc.tile_pool(name="wstream", bufs=4) as wpool,
            tc.tile_pool(name="xc", bufs=2) as xcpool,
            tc.tile_pool(name="stg", bufs=2) as stg,
            tc.tile_pool(name="epool", bufs=6) as epool,
            tc.tile_pool(name="odrain", bufs=4) as odrain,
            tc.tile_pool(name="bcast", bufs=1) as bcpool,
            tc.tile_pool(name="small", bufs=2) as smallp,
            tc.tile_pool(name="wostream", bufs=6) as wopool,
            tc.tile_pool(name="proj_ps", bufs=2, space=bass.MemorySpace.PSUM) as proj_ps,
            tc.tile_pool(name="s_ps", bufs=2, space=bass.MemorySpace.PSUM) as s_ps,
            tc.tile_pool(name="o_ps", bufs=2, space=bass.MemorySpace.PSUM) as o_ps,
            tc.tile_pool(name="misc_ps", bufs=1, space=bass.MemorySpace.PSUM) as misc_ps,
        ):
            # ---- persistent activations (declared first so DMAs below can
            # target them) ----
            qkT = persist.tile([128, Q_PER_CORE + KV_PER_CORE, S], BF16)
            vt = persist.tile([128, NKB, KVDIM], BF16)
            attn = persist.tile([128, Q_PER_CORE, S], BF16)

            # ---- constants ----
            cs_t = constp.tile([128, S], F32)
            sn_t = constp.tile([128, S], F32)
            mask_t = constp.tile([128, 128], BF16)
            ident_t = constp.tile([128, 128], BF16)
            ones_t = constp.tile([128, 1], BF16)
            nc.gpsimd.dma_start(out=cs_t[:], in_=cs[:])
            nc.gpsimd.dma_start(out=sn_t[:], in_=sn[:])
            nc.gpsimd.dma_start(out=mask_t[:], in_=masks[:])
            nc.gpsimd.dma_start(out=ident_t[:], in_=ident[:])
            nc.vector.memset(ones_t[:], 1.0)

            def emit_c_group(cc, ot):
                # one output-projection tile for chunk cc: 4 MMs + drain + DMA
                cc0, cc1 = cc * SCH, (cc + 1) * SCH
                wot = wopool.tile([128, Q_PER_CORE, 128], BF16, tag="wot")
                nc.gpsimd.dma_start(out=wot[:], in_=wo[ot])
                pso = proj_ps.tile([128, SCH], F32, tag="mm")
                for d in range(Q_PER_CORE):
                    nc.tensor.matmul(pso[:], wot[:, d, :], attn[:, d, cc0:cc1],
                                     start=(d == 0), stop=(d == Q_PER_CORE - 1))
                od = odrain.tile([128, SCH], BF16, tag="od")
                nc.vector.tensor_copy(od[:], pso[:])
                nc.sync.dma_start(out=out[ot * 128:(ot + 1) * 128, cc0:cc1],
                                  in_=od[:])

            for c in range(NSC):
                c0, c1 = c * SCH, (c + 1) * SCH

                # ============ A(c): QKV projection + RoPE + V transpose
                xc = xcpool.tile([128, KT, SCH], BF16)
                for piece in range(4):
                    kt0 = piece * (KT // 4)
                    kt1 = kt0 + KT // 4
                    nc.gpsimd.dma_start(
                        out=xc[:, kt0:kt1, :],
                        in_=xT[c, :, kt0:kt1, :])
                # m-tile order: k0 and q0 first so attention on head 0 can
                # start early; weight DMAs issue from the near-empty gpsimd
                # stream so they never queue behind exp/rope work
                morder = [4, 0, 5, 6, 7, 1, 2, 3]
                wts = {}
                for m in morder[:3]:
                    wt = wpool.tile([128, KT, 128], BF16, tag="wt")
                    nc.gpsimd.dma_start(out=wt[:], in_=wqkv[m])
                    wts[m] = wt
                for mi, m in enumerate(morder):
                    if mi + 3 < M_TILES:
                        mpre = morder[mi + 3]
                        wt = wpool.tile([128, KT, 128], BF16, tag="wt")
                        nc.gpsimd.dma_start(out=wt[:], in_=wqkv[mpre])
                        wts[mpre] = wt
                    wt = wts[m]
                    ps = proj_ps.tile([128, SCH], F32, tag="mm")
                    for k in range(KT):
                        nc.tensor.matmul(ps[:], wt[:, k, :], xc[:, k, :],
                                         start=(k == 0), stop=(k == KT - 1))
                    if m < Q_PER_CORE + KV_PER_CORE:
                        # RoPE: rot = T*cs + swap(T*sn)
                        t1 = stg.tile([128, SCH], F32, tag="t1")
                        t2 = stg.tile([128, SCH], F32, tag="t2")
                        t2s = stg.tile([128, SCH], F32, tag="t2s")
                        nc.vector.tensor_mul(t1[:], ps[:], cs_t[:, c0:c1])
                        nc.vector.tensor_mul(t2[:], ps[:], sn_t[:, c0:c1])
                        nc.sync.dma_start(out=t2s[0:64, :], in_=t2[64:128, :])
                        nc.sync.dma_start(out=t2s[64:128, :], in_=t2[0:64, :])
                        nc.vector.tensor_add(qkT[:, m, c0:c1], t1[:], t2s[:])
                    else:
                        # V: drain to bf16, then PE-transpose 128x128 blocks
                        kv = m - Q_PER_CORE - KV_PER_CORE
                        vtmp = stg.tile([128, SCH], BF16, tag="vtmp")
                        nc.scalar.activation(vtmp[:], ps[:],
                                             mybir.ActivationFunctionType.Copy)
                        for b in range(SCH // 128):
                            pt = misc_ps.tile([128, 128], BF16, tag="tp")
                            nc.tensor.transpose(pt[:], vtmp[:, b * 128:(b + 1) * 128],
                                                ident_t[:])
                            nc.vector.tensor_copy(
                                vt[:, c * (SCH // 128) + b, kv * 128:(kv + 1) * 128],
                                pt[:])
                    # interleave C(c-1) groups between the long projection
                    # groups so C's drain latency hides behind proj matmuls
                    if c > 0:
                        for j in range(NOT // M_TILES):
                            emit_c_group(c - 1, mi * (NOT // M_TILES) + j)

                # ============ B(c): attention for all 4 heads on q-chunk c
                nkb = (c + 1) * (SCH // 128)  # causal: k-blocks 0..4c+3
                for h in range(Q_PER_CORE):
                    kv = h // 2
                    kvm = Q_PER_CORE + kv
                    po = o_ps.tile([128, SCH], F32)
                    pz = misc_ps.tile([1, SCH], F32, tag="z")
                    es = [None] * nkb
                    # diagonal blocks only touch q-columns >= r0 = 128*off
                    # (columns below r0 are fully causally masked)
                    r0s = [max(0, 128 * (kb - (nkb - (SCH // 128)))) for kb in range(nkb)]
                    # software-pipelined: scores/exp run two k-blocks ahead of PV/Z
                    DEPTH = 2

                    def pv(kb):
                        r0 = r0s[kb]
                        nc.tensor.matmul(po[:, r0:SCH],
                                         vt[:, kb, kv * 128:(kv + 1) * 128],
                                         es[kb][:, r0:SCH],
                                         start=(kb == 0), stop=(kb == nkb - 1),
                                         skip_group_check=True)
                        nc.tensor.matmul(pz[:, r0:], ones_t[:], es[kb][:, r0:SCH],
                                         start=(kb == 0), stop=(kb == nkb - 1),
                                         skip_group_check=True)
                        es[kb] = None

                    for kb in range(nkb):
                        r0 = r0s[kb]
                        pss = s_ps.tile([128, SCH], F32)
                        nc.tensor.matmul(pss[:, r0:SCH],
                                         qkT[:, kvm, kb * 128:(kb + 1) * 128],
                                         qkT[:, h, c0 + r0:c1],
                                         start=True, stop=True)
                        e = epool.tile([128, SCH], BF16)
                        nc.scalar.activation(e[:, r0:], pss[:, r0:SCH],
                                             mybir.ActivationFunctionType.Exp,
                                             scale=SCALE)
                        if kb >= nkb - (SCH // 128):
                            # triangular mask on the diagonal 128-col sub-block
                            nc.vector.tensor_mul(e[:, r0:r0 + 128],
                                                 e[:, r0:r0 + 128], mask_t[:])
                        es[kb] = e
                        if kb >= DEPTH:
                            pv(kb - DEPTH)
                    for kb in range(max(0, nkb - DEPTH), nkb):
                        pv(kb)
                    # drain quickly; normalization is deferred off the PE path
                    nc.scalar.activation(attn[:, h, c0:c1], po[:],
                                         mybir.ActivationFunctionType.Copy)
                    zrow = smallp.tile([1, SCH], F32, tag="zrow")
                    nc.scalar.activation(zrow[:], pz[:],
                                         mybir.ActivationFunctionType.Copy)
                    rec = smallp.tile([1, SCH], F32, tag="rec")
                    nc.vector.reciprocal_approx_fast(rec[:], zrow[:])
                    bc = bcpool.tile([128, SCH], F32)
                    nc.gpsimd.partition_broadcast(bc[:], rec[:])
                    nc.vector.tensor_mul(attn[:, h, c0:c1], attn[:, h, c0:c1], bc[:])

            # final chunk's output projection has no later phase to hide in
            for ot in range(NOT):
                emit_c_group(NSC - 1, ot)

    nc.compile()
    return nc


def _host_prep(x, freqs_cis, wq, wk, wv, wo, start_pos):
    """Build per-core in_maps (host-side shard + layout transform)."""
    sp = int(start_pos)
    x2 = np.asarray(x, np.float32).reshape(S, EMBED)
    fc = np.asarray(freqs_cis, np.float32)[sp:sp + S]  # [S, 64, 2]
    cos = fc[:, :, 0].T.copy()  # [64, S]
    sin = fc[:, :, 1].T.copy()
    cs = np.concatenate([cos, cos], axis=0).astype(np.float32)          # [128,S]
    sn = np.concatenate([sin, -sin], axis=0).astype(np.float32)         # [128,S]

    # de-interleave perm within each head (evens then odds)
    perm = np.concatenate([np.arange(0, HD, 2), np.arange(1, HD, 2)])
    qperm = np.concatenate([h * HD + perm for h in range(HEADS)])
    kperm = np.concatenate([h * HD + perm for h in range(KV_HEADS)])

    wq_p = np.asarray(wq, np.float32)[:, qperm]
    wk_p = np.asarray(wk, np.float32)[:, kperm]
    wv_p = np.asarray(wv, np.float32)
    wo_p = np.asarray(wo, np.float32)

    xT = np.ascontiguousarray(x2.T).astype(NPBF16)        # [EMBED, S]
    # chunk-major: [NSC, 128, KT*SCH]
    xT4 = np.ascontiguousarray(
        xT.reshape(KT, 128, NSC, SCH).transpose(2, 1, 0, 3))

    kk = np.arange(128)[:, None]
    qq = np.arange(128)[None, :]
    m4 = (qq >= kk).astype(NPBF16)  # [128,128] lower-triangular in q>=k sense

    ident = np.eye(128, dtype=NPBF16)

    in_maps = []
    for c in range(N_CORES):
        wq_c = wq_p[:, c * QDIM:(c + 1) * QDIM]
        wk_c = wk_p[:, c * KVDIM:(c + 1) * KVDIM]
        wv_c = wv_p[:, c * KVDIM:(c + 1) * KVDIM]
        wcat = np.concatenate([wq_c, wk_c, wv_c], axis=1)  # [EMBED, 1024]
        # m-tile-major: [8, 128, KT*128]
        wqkv = np.ascontiguousarray(
            wcat.reshape(KT, 128, M_TILES, 128).transpose(2, 1, 0, 3)
        ).astype(NPBF16)
        wo_c = wo_p[c * QDIM:(c + 1) * QDIM, :]            # [512, EMBED]
        # ot-major: [NOT, 128, QDIM] with [d' within head (128), head (4)] per ot
        woL = np.ascontiguousarray(
            wo_c.reshape(Q_PER_CORE, 128, NOT, 128).transpose(2, 1, 0, 3)
        ).astype(NPBF16)
        in_maps.append({
            "xT": xT4,
            "wqkv": wqkv,
            "wo": woL,
            "cs": cs, "sn": sn, "masks": m4, "ident": ident,
        })
    return in_maps


def kernel(x, freqs_cis, wq, wk, wv, wo, start_pos):
    if "nc" not in _cache:
        _cache["nc"] = _build()
    nc = _cache["nc"]
    in_maps = _host_prep(x, freqs_cis, wq, wk, wv, wo, start_pos)
    res = run_bass_kernel_spmd(nc, in_maps, core_ids=list(range(N_CORES)))
    parts = [res.results[c]["out"] for c in range(N_CORES)]
    total = parts[0].astype(np.float32)
    for p in parts[1:]:
        total = total + p.astype(np.float32)
    return np.ascontiguousarray(total.T).reshape(1, S, EMBED).astype(np.float32)


# revision 42
# speedup vs baseline: 1.2177x; 1.0121x over previous
"""Tensor-parallel GQA attention (RoPE + causal softmax) on 8 TRN2 NeuronCores.

Sharding: wq/wk/wv column-sharded by head (4 q heads + 2 kv heads per core),
wo row-sharded; x replicated. Each core computes a partial [4096, 2048]
(transposed) output; the host sums the 8 partials and transposes.

All on-device layouts keep the contraction dim on SBUF partitions:
  - projections:  psum[m,s]   = sum_k  W[k,m].T   @ xT[k,s]      (q/k/v in ^T layout)
  - RoPE: host de-interleaves wq/wk columns per head (evens then odds) so the
    rotation becomes full-lane DVE ops + one SBUF->SBUF partition-half swap DMA.
  - scores^T[k,q] = kT[d,k].T @ qT[d,q];  E = exp(scale*s) on ACT, causal mask
    as a 0/1 multiply on (diagonal) partial blocks only.
  - out^T[d',q] += v[k,d'].T @ E[k,q]  accumulated over k-blocks (PSUM), with a
    parallel ones-stationary matmul producing Z[1,q]; normalization is deferred
    off the critical path (fast reciprocal + partition broadcast + in-place mul).
  - final^T[o,q] += wo[d',o].T @ attnT[d',q].

The whole kernel is chunk-pipelined: for each 512-token chunk c,
A(c) projection -> B(c) attention -> C(c) output projection, so the three
phases overlap across chunks.
"""

import numpy as np
import ml_dtypes

import concourse.bacc as bacc
import concourse.bass as bass
import concourse.mybir as mybir
import concourse.tile as tile
from concourse.bass_utils import run_bass_kernel_spmd

BF16 = mybir.dt.bfloat16
F32 = mybir.dt.float32
NPBF16 = ml_dtypes.bfloat16

EMBED = 4096
S = 2048
HEADS = 32
KV_HEADS = 16
HD = 128
N_CORES = 8
Q_PER_CORE = HEADS // N_CORES          # 4
KV_PER_CORE = KV_HEADS // N_CORES      # 2
QDIM = Q_PER_CORE * HD                 # 512
KVDIM = KV_PER_CORE * HD               # 256
KT = EMBED // 128                      # 32 contraction k-tiles
SCH = 512                              # s-chunk (matmul moving cols / PSUM bank)
NSC = S // SCH                         # 4 s-chunks
NKB = S // 128                         # 16 k-blocks
M_TILES = Q_PER_CORE + KV_PER_CORE + KV_PER_CORE  # 8 projection m-tiles
NOT = EMBED // 128                     # 32 output tiles
SCALE = 1.0 / float(np.sqrt(HD))

_cache = {}


def _build():
    nc = bacc.Bacc("TRN2", target_bir_lowering=False, debug=False,
                   num_devices=N_CORES)

    # m-tile-major / chunk-major layouts so every DMA is contiguous per partition
    xT = nc.dram_tensor("xT", [NSC, 128, KT, SCH], BF16, kind="ExternalInput")
    wqkv = nc.dram_tensor("wqkv", [M_TILES, 128, KT, 128], BF16, kind="ExternalInput")
    wo = nc.dram_tensor("wo", [NOT, 128, Q_PER_CORE, 128], BF16, kind="ExternalInput")
    cs = nc.dram_tensor("cs", [128, S], F32, kind="ExternalInput")
    sn = nc.dram_tensor("sn", [128, S], F32, kind="ExternalInput")
    masks = nc.dram_tensor("masks", [128, 128], BF16, kind="ExternalInput")
    ident = nc.dram_tensor("ident", [128, 128], BF16, kind="ExternalInput")
    out = nc.dram_tensor("out", [EMBED, S], BF16, kind="ExternalOutput")

    with tile.TileContext(nc) as tc:
        with (
            tc.tile_pool(name="const", bufs=1) as constp,
            tc.tile_pool(name="persist", bufs=1) as persist,
            tc.tile_pool(name="wstream", bufs=4) as wpool,
            tc.tile_pool(name="xc", bufs=2) as xcpool,
            tc.tile_pool(name="stg", bufs=2) as stg,
            tc.tile_pool(name="epool", bufs=6) as epool,
            tc.tile_pool(name="odrain", bufs=4) as odrain,
            tc.tile_pool(name="bcast", bufs=1) as bcpool,
            tc.tile_pool(name="small", bufs=2) as smallp,
            tc.tile_pool(name="wostream", bufs=6) as wopool,
            tc.tile_pool(name="proj_ps", bufs=2, space=bass.MemorySpace.PSUM) as proj_ps,
            tc.tile_pool(name="s_ps", bufs=2, space=bass.MemorySpace.PSUM) as s_ps,
            tc.tile_pool(name="o_ps", bufs=2, space=bass.MemorySpace.PSUM) as o_ps,
            tc.tile_pool(name="misc_ps", bufs=1, space=bass.MemorySpace.PSUM) as misc_ps,
        ):
            # ---- persistent activations (declared first so DMAs below can
            # target them) ----
            qkT = persist.tile([128, Q_PER_CORE + KV_PER_CORE, S], BF16)
            vt = persist.tile([128, NKB, KVDIM], BF16)
            attn = persist.tile([128, Q_PER_CORE, S], BF16)

            # ---- constants ----
            cs_t = constp.tile([128, S], F32)
            sn_t = constp.tile([128, S], F32)
            mask_t = constp.tile([128, 128], BF16)
            ident_t = constp.tile([128, 128], BF16)
            ones_t = constp.tile([128, 1], BF16)
            nc.vector.memset(ones_t[:], 1.0)

            def emit_consts():
                # emitted after the first chunk's data DMAs so they don't
                # delay the first matmuls; sync queue is idle at startup
                nc.sync.dma_start(out=cs_t[:], in_=cs[:])
                nc.sync.dma_start(out=sn_t[:], in_=sn[:])
                nc.sync.dma_start(out=mask_t[:], in_=masks[:])
                nc.sync.dma_start(out=ident_t[:], in_=ident[:])

            def emit_c_group(cc, ot):
                # one output-projection tile for chunk cc: 4 MMs + drain + DMA
                cc0, cc1 = cc * SCH, (cc + 1) * SCH
                wot = wopool.tile([128, Q_PER_CORE, 128], BF16, tag="wot")
                nc.gpsimd.dma_start(out=wot[:], in_=wo[ot])
                pso = proj_ps.tile([128, SCH], F32, tag="mm")
                for d in range(Q_PER_CORE):
                    nc.tensor.matmul(pso[:], wot[:, d, :], attn[:, d, cc0:cc1],
                                     start=(d == 0), stop=(d == Q_PER_CORE - 1))
                od = odrain.tile([128, SCH], BF16, tag="od")
                nc.vector.tensor_copy(od[:], pso[:])
                nc.sync.dma_start(out=out[ot * 128:(ot + 1) * 128, cc0:cc1],
                                  in_=od[:])

            for c in range(NSC):
                c0, c1 = c * SCH, (c + 1) * SCH

                # ============ A(c): QKV projection + RoPE + V transpose
                xc = xcpool.tile([128, KT, SCH], BF16)
                for piece in range(4):
                    kt0 = piece * (KT // 4)
                    kt1 = kt0 + KT // 4
                    nc.gpsimd.dma_start(
                        out=xc[:, kt0:kt1, :],
                        in_=xT[c, :, kt0:kt1, :])
                # m-tile order: k0 and q0 first so attention on head 0 can
                # start early; weight DMAs issue from the near-empty gpsimd
                # stream so they never queue behind exp/rope work
                morder = [4, 0, 5, 6, 7, 1, 2, 3]
                wts = {}
                for m in morder[:3]:
                    wt = wpool.tile([128, KT, 128], BF16, tag="wt")
                    nc.gpsimd.dma_start(out=wt[:], in_=wqkv[m])
                    wts[m] = wt
                if c == 0:
                    emit_consts()
                for mi, m in enumerate(morder):
                    if mi + 3 < M_TILES:
                        mpre = morder[mi + 3]
                        wt = wpool.tile([128, KT, 128], BF16, tag="wt")
                        nc.gpsimd.dma_start(out=wt[:], in_=wqkv[mpre])
                        wts[mpre] = wt
                    wt = wts[m]
                    ps = proj_ps.tile([128, SCH], F32, tag="mm")
                    for k in range(KT):
                        nc.tensor.matmul(ps[:], wt[:, k, :], xc[:, k, :],
                                         start=(k == 0), stop=(k == KT - 1))
                    if m < Q_PER_CORE + KV_PER_CORE:
                        # RoPE: rot = T*cs + swap(T*sn). One fast ACT copy
                        # frees the PSUM slot; DVE muls then read SBUF.
                        tsb = stg.tile([128, SCH], F32, tag="tsb")
                        nc.scalar.activation(tsb[:], ps[:],
                                             mybir.ActivationFunctionType.Copy)
                        t1 = stg.tile([128, SCH], F32, tag="t1")
                        t2 = stg.tile([128, SCH], F32, tag="t2")
                        t2s = stg.tile([128, SCH], F32, tag="t2s")
                        nc.vector.tensor_mul(t1[:], tsb[:], cs_t[:, c0:c1])
                        nc.vector.tensor_mul(t2[:], tsb[:], sn_t[:, c0:c1])
                        nc.sync.dma_start(out=t2s[0:64, :], in_=t2[64:128, :])
                        nc.sync.dma_start(out=t2s[64:128, :], in_=t2[0:64, :])
                        nc.vector.tensor_add(qkT[:, m, c0:c1], t1[:], t2s[:])
                    else:
                        # V: drain to bf16, then PE-transpose 128x128 blocks
                        kv = m - Q_PER_CORE - KV_PER_CORE
                        vtmp = stg.tile([128, SCH], BF16, tag="vtmp")
                        nc.scalar.activation(vtmp[:], ps[:],
                                             mybir.ActivationFunctionType.Copy)
                        for b in range(SCH // 128):
                            pt = misc_ps.tile([128, 128], BF16, tag="tp")
                            nc.tensor.transpose(pt[:], vtmp[:, b * 128:(b + 1) * 128],
                                                ident_t[:])
                            nc.vector.tensor_copy(
                                vt[:, c * (SCH // 128) + b, kv * 128:(kv + 1) * 128],
                                pt[:])
                    # interleave C(c-1) groups between the long projection
                    # groups so C's drain latency hides behind proj matmuls
                    if c > 0:
                        for j in range(NOT // M_TILES):
                            emit_c_group(c - 1, mi * (NOT // M_TILES) + j)

                # ============ B(c): attention for all 4 heads on q-chunk c
                nkb = (c + 1) * (SCH // 128)  # causal: k-blocks 0..4c+3
                for h in range(Q_PER_CORE):
                    kv = h // 2
                    kvm = Q_PER_CORE + kv
                    po = o_ps.tile([128, SCH], F32)
                    pz = misc_ps.tile([1, SCH], F32, tag="z")
                    es = [None] * nkb
                    # diagonal blocks only touch q-columns >= r0 = 128*off
                    # (columns below r0 are fully causally masked)
                    r0s = [max(0, 128 * (kb - (nkb - (SCH // 128)))) for kb in range(nkb)]
                    # software-pipelined: scores/exp run two k-blocks ahead of PV/Z
                    DEPTH = 2

                    def pv(kb):
                        r0 = r0s[kb]
                        nc.tensor.matmul(po[:, r0:SCH],
                                         vt[:, kb, kv * 128:(kv + 1) * 128],
                                         es[kb][:, r0:SCH],
                                         start=(kb == 0), stop=(kb == nkb - 1),
                                         skip_group_check=True)
                        nc.tensor.matmul(pz[:, r0:], ones_t[:], es[kb][:, r0:SCH],
                                         start=(kb == 0), stop=(kb == nkb - 1),
                                         skip_group_check=True)
                        es[kb] = None

                    for kb in range(nkb):
                        r0 = r0s[kb]
                        pss = s_ps.tile([128, SCH], F32)
                        nc.tensor.matmul(pss[:, r0:SCH],
                                         qkT[:, kvm, kb * 128:(kb + 1) * 128],
                                         qkT[:, h, c0 + r0:c1],
                                         start=True, stop=True)
                        e = epool.tile([128, SCH], BF16)
                        nc.scalar.activation(e[:, r0:], pss[:, r0:SCH],
                                             mybir.ActivationFunctionType.Exp,
                                             scale=SCALE)
                        if kb >= nkb - (SCH // 128):
                            # triangular mask on the diagonal 128-col sub-block
                            nc.vector.tensor_mul(e[:, r0:r0 + 128],
                                                 e[:, r0:r0 + 128], mask_t[:])
                        es[kb] = e
                        if kb >= DEPTH:
                            pv(kb - DEPTH)
                    for kb in range(max(0, nkb - DEPTH), nkb):
                        pv(kb)
                    # drain quickly; normalization is deferred off the PE path
                    nc.scalar.activation(attn[:, h, c0:c1], po[:],
                                         mybir.ActivationFunctionType.Copy)
                    zrow = smallp.tile([1, SCH], F32, tag="zrow")
                    nc.scalar.activation(zrow[:], pz[:],
                                         mybir.ActivationFunctionType.Copy)
                    rec = smallp.tile([1, SCH], F32, tag="rec")
                    nc.vector.reciprocal_approx_fast(rec[:], zrow[:])
                    bc = bcpool.tile([128, SCH], F32)
                    nc.gpsimd.partition_broadcast(bc[:], rec[:])
                    nc.vector.tensor_mul(attn[:, h, c0:c1], attn[:, h, c0:c1], bc[:])

            # final chunk's output projection has no later phase to hide in
            for ot in range(NOT):
                emit_c_group(NSC - 1, ot)

    nc.compile()
    return nc


def _host_prep(x, freqs_cis, wq, wk, wv, wo, start_pos):
    """Build per-core in_maps (host-side shard + layout transform)."""
    sp = int(start_pos)
    x2 = np.asarray(x, np.float32).reshape(S, EMBED)
    fc = np.asarray(freqs_cis, np.float32)[sp:sp + S]  # [S, 64, 2]
    cos = fc[:, :, 0].T.copy()  # [64, S]
    sin = fc[:, :, 1].T.copy()
    cs = np.concatenate([cos, cos], axis=0).astype(np.float32)          # [128,S]
    sn = np.concatenate([sin, -sin], axis=0).astype(np.float32)         # [128,S]

    # de-interleave perm within each head (evens then odds)
    perm = np.concatenate([np.arange(0, HD, 2), np.arange(1, HD, 2)])
    qperm = np.concatenate([h * HD + perm for h in range(HEADS)])
    kperm = np.concatenate([h * HD + perm for h in range(KV_HEADS)])

    wq_p = np.asarray(wq, np.float32)[:, qperm]
    wk_p = np.asarray(wk, np.float32)[:, kperm]
    wv_p = np.asarray(wv, np.float32)
    wo_p = np.asarray(wo, np.float32)

    xT = np.ascontiguousarray(x2.T).astype(NPBF16)        # [EMBED, S]
    # chunk-major: [NSC, 128, KT*SCH]
    xT4 = np.ascontiguousarray(
        xT.reshape(KT, 128, NSC, SCH).transpose(2, 1, 0, 3))

    kk = np.arange(128)[:, None]
    qq = np.arange(128)[None, :]
    m4 = (qq >= kk).astype(NPBF16)  # [128,128] lower-triangular in q>=k sense

    ident = np.eye(128, dtype=NPBF16)

    in_maps = []
    for c in range(N_CORES):
        wq_c = wq_p[:, c * QDIM:(c + 1) * QDIM]
        wk_c = wk_p[:, c * KVDIM:(c + 1) * KVDIM]
        wv_c = wv_p[:, c * KVDIM:(c + 1) * KVDIM]
        wcat = np.concatenate([wq_c, wk_c, wv_c], axis=1)  # [EMBED, 1024]
        # m-tile-major: [8, 128, KT*128]
        wqkv = np.ascontiguousarray(
            wcat.reshape(KT, 128, M_TILES, 128).transpose(2, 1, 0, 3)
        ).astype(NPBF16)
        wo_c = wo_p[c * QDIM:(c + 1) * QDIM, :]            # [512, EMBED]
        # ot-major: [NOT, 128, QDIM] with [d' within head (128), head (4)] per ot
        woL = np.ascontiguousarray(
            wo_c.reshape(Q_PER_CORE, 128, NOT, 128).transpose(2, 1, 0, 3)
        ).astype(NPBF16)
        in_maps.append({
            "xT": xT4,
            "wqkv": wqkv,
            "wo": woL,
            "cs": cs, "sn": sn, "masks": m4, "ident": ident,
        })
    return in_maps


def kernel(x, freqs_cis, wq, wk, wv, wo, start_pos):
    if "nc" not in _cache:
        _cache["nc"] = _build()
    nc = _cache["nc"]
    in_maps = _host_prep(x, freqs_cis, wq, wk, wv, wo, start_pos)
    res = run_bass_kernel_spmd(nc, in_maps, core_ids=list(range(N_CORES)))
    parts = [res.results[c]["out"] for c in range(N_CORES)]
    total = parts[0].astype(np.float32)
    for p in parts[1:]:
        total = total + p.astype(np.float32)
    return np.ascontiguousarray(total.T).reshape(1, S, EMBED).astype(np.float32)
